# revision 5
# baseline (speedup 1.0000x reference)
"""CrossAttention Trainium2 kernel (8 NeuronCores, SPMD), bf16 compute.

Sharding: data-parallel over batch B=2, tensor-parallel over the 16 heads in
4 groups of 4 heads -> 8 cores, one (batch, head-group) pair each. Each core
computes its 4 heads' Q/K/V projections, masked softmax cross-attention, and
its partial output projection; the host sums the 4 partial outputs per batch
(the Wo row-split all-reduce, done at unshard time) and adds the constant
Wo @ b_v term (softmax rows sum to 1 so it factors out of the attention).

Numerics: bf16 matmuls with fp32 PSUM accumulation; softmax statistics stay
fp32. Masking is additive before exp: maskbias = (mask ? 0 : -24) stored in
fp8e4m3 (both values exact, 1 byte/elem), applied by a DVE
scalar_tensor_tensor that also applies the softmax scale. y is written bf16;
the host accumulates the partials in fp32.

Layout: activations and weights arrive contraction-major (host
pre-transposed) so every DMA is a contiguous row load; no device transposes.
Attention is scores-transposed: ST[m, n] per head, so PV contracts over m
directly. The softmax denominator comes free from an appended ones-column on
the v stationary operand. Masked exps live in a 4-deep rotating buffer: PV
consumes them one m-tile behind the exp, so nothing is parked.

Schedule: two n-half passes over the 16 m-tiles (PSUM: 4 score banks + 4 PV
accumulator banks per pass). Pass 0 interleaves the V projection and the
last 3/4 of the K projection into the per-m-tile flow, so only Q and the
first K chunk gate the pipeline start. Pass 0's normalize + output
projection + y writeback overlap pass 1; odd-head normalized outputs reach
partitions 64:128 via an identity-stationary PE matmul at column offset 64
(cheaper than the SBUF-SBUF shift DMA on the critical tail).
"""

import numpy as np
import ml_dtypes

import concourse.bass as bass
import concourse.bacc as bacc
import concourse.mybir as mybir
import concourse.tile as tile
from concourse.bass_utils import run_bass_kernel_spmd

DIM = 1024
HEAD_DIM = 64
NUM_HEADS = 16
SCALE = HEAD_DIM**-0.5
B, N, M = 2, 1024, 2048
HPC = 4  # heads per core
E = HPC * HEAD_DIM  # 256: per-core projection width
P = 128
F32 = mybir.dt.float32
BF16 = mybir.dt.bfloat16
F8 = mybir.dt.float8e4
CT = DIM // P  # 8 contraction tiles
MT = M // P  # 16 m tiles
MBIAS = -24.0  # additive mask bias (pre-exp); exact in e4m3


def _bc2(ap):
    """Broadcast a [P, F] AP to [P, 2, F] with a zero-stride middle dim."""
    return bass.AP(ap.tensor, ap.offset, [ap.ap[0], [0, 2], ap.ap[1]])


def _group_heads(ap, hpc, hd):
    """View a [P, hpc*hd] AP as [P, hpc, hd]."""
    assert ap.ap[-1][0] == 1 and ap.ap[-1][1] == hpc * hd
    return bass.AP(ap.tensor, ap.offset, [ap.ap[0], [hd, hpc], [1, hd]])


def build_program():
    nc = bacc.Bacc("TRN2", target_bir_lowering=False, debug=False, num_devices=8)

    # contraction-major inputs (host pre-transposed)
    xT_d = nc.dram_tensor("xT", [DIM, N], BF16, kind="ExternalInput").ap()
    ctxT_d = nc.dram_tensor("ctxT", [DIM, M], BF16, kind="ExternalInput").ap()
    mb_d = nc.dram_tensor("mb", [M, N], F8, kind="ExternalInput").ap()
    wqT_d = nc.dram_tensor("wqT", [DIM, E], BF16, kind="ExternalInput").ap()
    wkT_d = nc.dram_tensor("wkT", [DIM, E], BF16, kind="ExternalInput").ap()
    wvT_d = nc.dram_tensor("wvT", [DIM, E], BF16, kind="ExternalInput").ap()
    woT_d = nc.dram_tensor("woT", [E, DIM], BF16, kind="ExternalInput").ap()
    bk_d = nc.dram_tensor("bk", [E], F32, kind="ExternalInput").ap()
    eye_d = nc.dram_tensor("eye64", [HEAD_DIM, HEAD_DIM], BF16, kind="ExternalInput").ap()
    y_d = nc.dram_tensor("y", [N, DIM], BF16, kind="ExternalOutput").ap()

    Exp = mybir.ActivationFunctionType.Exp
    Mult = mybir.AluOpType.mult
    Add = mybir.AluOpType.add

    from contextlib import ExitStack

    with tile.TileContext(nc) as tc, ExitStack() as ctx:
        const = ctx.enter_context(tc.tile_pool(name="const", bufs=1))
        bk_sb = const.tile([P, E // P], F32)
        eye64 = const.tile([HEAD_DIM, HEAD_DIM], BF16)
        nc.gpsimd.dma_start(out=bk_sb, in_=bk_d.rearrange("(t p) -> p t", p=P))
        nc.gpsimd.dma_start(out=eye64, in_=eye_d)

        persist = ctx.enter_context(tc.tile_pool(name="persist", bufs=1))
        qT = persist.tile([P, E // P, N], BF16)
        kT = persist.tile([P, E // P, M], BF16)
        vaug = persist.tile([P, MT, HPC, HEAD_DIM + 1], BF16)
        woT = persist.tile([P, E // P, DIM], BF16)
        mbias = persist.tile([P, MT, N], F8)
        ot_sb = persist.tile([HEAD_DIM + 1, HPC, N], F32)  # PV accumulator park
        otn2 = persist.tile([P, E // P, N], BF16)  # normalized attn out

        # ones column for the softmax denominator; v evictions fill 0:64
        nc.vector.memset(vaug, 1.0)

        stgp = ctx.enter_context(tc.tile_pool(name="stgp", bufs=3))
        expl = ctx.enter_context(tc.tile_pool(name="expl", bufs=4))
        dnp = ctx.enter_context(tc.tile_pool(name="dnp", bufs=2))
        rbp = ctx.enter_context(tc.tile_pool(name="rbp", bufs=3))
        ypool = ctx.enter_context(tc.tile_pool(name="ypool", bufs=3))

        ex_tiles = {}

        def emit_scores(spool, mt, chn, hp):
            """bf16 scores + DVE scale/maskadd + ACT exp -> rotating ex tile."""
            st = spool.tile([P, 2, 512], F32, tag="st", name="st", bufs=1)
            for hl in range(2):
                h = hp * 2 + hl
                dr = slice(hl * HEAD_DIM, (hl + 1) * HEAD_DIM)
                nc.tensor.matmul(
                    st[:, hl, :],
                    lhsT=kT[dr, hp, mt * P : (mt + 1) * P],
                    rhs=qT[dr, hp, chn * 512 : (chn + 1) * 512],
                    start=True,
                    stop=True,
                )
            stg = stgp.tile([P, 2, 512], BF16, tag="stg", name="stg")
            mk = _bc2(mbias[:, mt, chn * 512 : (chn + 1) * 512])
            nc.vector.scalar_tensor_tensor(
                out=stg, in0=st, scalar=float(SCALE), in1=mk, op0=Mult, op1=Add
            )
            ex = expl.tile([P, 2, 512], BF16, tag="ex", name="ex")
            nc.scalar.activation(ex, stg, Exp)
            ex_tiles[(mt, chn, hp)] = ex

        def emit_pv(ot_ps, mt, chn):
            for hp in range(2):
                ex = ex_tiles[(mt, chn, hp)]
                for hl in range(2):
                    h = hp * 2 + hl
                    nc.tensor.matmul(
                        ot_ps[h],
                        lhsT=vaug[:, mt, h, :],
                        rhs=ex[:, hl, :],
                        start=(mt == 0),
                        stop=(mt == MT - 1),
                    )

        # ---------------- input DMAs ---------------------------------------
        with tc.tile_pool(name="wx", bufs=1) as wx_pool:
            wqT = wx_pool.tile([P, CT, E], BF16)
            xT = wx_pool.tile([P, CT, N], BF16)
            wkT = wx_pool.tile([P, CT, E], BF16)
            wvT = wx_pool.tile([P, CT, E], BF16)
            ctxT = wx_pool.tile([P, CT, M], BF16)

            # sync ring: wq, x (Q path)
            nc.sync.dma_start(out=wqT, in_=wqT_d.rearrange("(c p) e -> p c e", p=P))
            for j in range(CT):
                nc.sync.dma_start(out=xT[:, j, :], in_=xT_d[j * P : (j + 1) * P, :])
            # scalar ring: wk, ctx first m-half, ctx second m-half j0-3
            nc.scalar.dma_start(out=wkT, in_=wkT_d.rearrange("(c p) e -> p c e", p=P))
            for j in range(CT):
                nc.scalar.dma_start(
                    out=ctxT[:, j, : M // 2], in_=ctxT_d[j * P : (j + 1) * P, : M // 2]
                )
            for j in range(4):
                nc.scalar.dma_start(
                    out=ctxT[:, j, M // 2 :], in_=ctxT_d[j * P : (j + 1) * P, M // 2 :]
                )
            # gpsimd ring: early mask, wv, ctx second m-half j4-7, mask, wo
            for mt in range(4):
                nc.gpsimd.dma_start(
                    out=mbias[:, mt, :], in_=mb_d[mt * P : (mt + 1) * P, :]
                )
            nc.gpsimd.dma_start(out=wvT, in_=wvT_d.rearrange("(c p) e -> p c e", p=P))
            for j in range(4, CT):
                nc.gpsimd.dma_start(
                    out=ctxT[:, j, M // 2 :], in_=ctxT_d[j * P : (j + 1) * P, M // 2 :]
                )
            for mt in range(4, MT):
                nc.gpsimd.dma_start(
                    out=mbias[:, mt, :], in_=mb_d[mt * P : (mt + 1) * P, :]
                )
            nc.gpsimd.dma_start(out=woT, in_=woT_d.rearrange("(c p) e -> p c e", p=P))

            # ---------------- Q projection (x-gated) -----------------------
            with tc.tile_pool(name="qps", bufs=2, space="PSUM") as qps:
                for et in range(E // P):
                    for chn in range(N // 512):
                        pq = qps.tile([P, 512], F32, tag="pq")
                        for j in range(CT):
                            nc.tensor.matmul(
                                pq,
                                lhsT=wqT[:, j, et * P : (et + 1) * P],
                                rhs=xT[:, j, chn * 512 : (chn + 1) * 512],
                                start=(j == 0),
                                stop=(j == CT - 1),
                            )
                        nc.vector.tensor_copy(qT[:, et, chn * 512 : (chn + 1) * 512], pq)

            # ---------------- K chunk 0, then pass 0 with V/K inline -------
            with (
                tc.tile_pool(name="kps", bufs=1, space="PSUM") as kps,
                tc.tile_pool(name="vps", bufs=1, space="PSUM") as vps,
            ):

                def emit_kproj(et, chm):
                    pk = kps.tile([P, 512], F32, tag="pk", name="pk")
                    for j in range(CT):
                        nc.tensor.matmul(
                            pk,
                            lhsT=wkT[:, j, et * P : (et + 1) * P],
                            rhs=ctxT[:, j, chm * 512 : (chm + 1) * 512],
                            start=(j == 0),
                            stop=(j == CT - 1),
                        )
                    nc.vector.tensor_scalar_add(
                        kT[:, et, chm * 512 : (chm + 1) * 512],
                        pk,
                        bk_sb[:, et : et + 1],
                    )

                def emit_vproj(mt):
                    pv = vps.tile([P, E], F32, tag="pv", name="pv")
                    for j in range(CT):
                        nc.tensor.matmul(
                            pv,
                            lhsT=ctxT[:, j, mt * P : (mt + 1) * P],
                            rhs=wvT[:, j, :],
                            start=(j == 0),
                            stop=(j == CT - 1),
                        )
                    nc.vector.tensor_copy(
                        vaug[:, mt, :, :HEAD_DIM],
                        _group_heads(pv[:, :], HPC, HEAD_DIM),
                    )

                emit_kproj(0, 0)
                emit_kproj(1, 0)
                emit_vproj(0)

                # pass 0: n-cols 0:512; V(mt+1) and K chm 1-3 ride along
                with (
                    tc.tile_pool(name="ops0", bufs=1, space="PSUM") as ops0,
                    tc.tile_pool(name="sps0", bufs=1, space="PSUM") as sps0,
                ):
                    ot_ps0 = [
                        ops0.tile([HEAD_DIM + 1, 512], F32, tag=f"o{h}", name=f"o{h}")
                        for h in range(HPC)
                    ]
                    kfill = {0: (0, 1), 1: (1, 1), 6: (0, 2), 7: (1, 2),
                             10: (0, 3), 11: (1, 3)}
                    for mt in range(MT):
                        emit_scores(sps0, mt, 0, 0)
                        if mt < MT - 1:
                            emit_vproj(mt + 1)
                        emit_scores(sps0, mt, 0, 1)
                        if mt > 0:
                            emit_pv(ot_ps0, mt - 1, 0)
                        if mt in kfill:
                            emit_kproj(*kfill[mt])
                    emit_pv(ot_ps0, MT - 1, 0)
                    for h in range(HPC):
                        nc.vector.tensor_copy(ot_sb[:, h, :512], ot_ps0[h])

        def normalize(h, chn, yps):
            """softmax-normalize head h's n-half chn from the ot_sb park."""
            cs = slice(chn * 512, (chn + 1) * 512)
            hp, hl = divmod(h, 2)
            dn0 = dnp.tile([1, 512], F32, tag="dn", name="dn")
            # row 64 (denominator) -> partition 0 via SBUF-SBUF DMA
            nc.sync.dma_start(out=dn0, in_=ot_sb[HEAD_DIM : HEAD_DIM + 1, h, cs])
            rc = rbp.tile([1, 512], F32, tag="rc", name="rc")
            nc.vector.reciprocal_approx_fast(out=rc, in_=dn0)
            rb = rbp.tile([HEAD_DIM, 512], F32, tag="rb", name="rb")
            nc.gpsimd.partition_broadcast(rb, rc)
            if hl == 0:
                nc.vector.tensor_mul(
                    otn2[:HEAD_DIM, hp, cs], ot_sb[:HEAD_DIM, h, cs], rb
                )
            else:
                # normalized out -> partitions 64:128 via identity matmul
                tmp = rbp.tile([HEAD_DIM, 512], BF16, tag="tmp", name="tmp")
                nc.vector.tensor_mul(tmp, ot_sb[:HEAD_DIM, h, cs], rb)
                sh = yps.tile([P, 512], F32, tag="sh", name="sh")
                nc.tensor.matmul(
                    sh[HEAD_DIM:P, :], lhsT=eye64, rhs=tmp, start=True, stop=True
                )
                nc.vector.tensor_copy(otn2[HEAD_DIM:P, hp, cs], sh[HEAD_DIM:P, :])

        def emit_oproj(yps, nb, ring):
            for oc in range(DIM // 512):
                yp = yps.tile([P, 512], F32, tag="yp", name="yp")
                for hp in range(E // P):
                    nc.tensor.matmul(
                        yp,
                        lhsT=otn2[:, hp, nb * P : (nb + 1) * P],
                        rhs=woT[:, hp, oc * 512 : (oc + 1) * 512],
                        start=(hp == 0),
                        stop=(hp == E // P - 1),
                    )
                ys = ypool.tile([P, 512], BF16, tag="ys", name="ys")
                nc.vector.tensor_copy(ys, yp)
                ring.dma_start(
                    out=y_d[nb * P : (nb + 1) * P, oc * 512 : (oc + 1) * 512], in_=ys
                )

        # ---------------- pass 1: n-cols 512:1024 + pass-0 tail work -------
        with (
            tc.tile_pool(name="ops1", bufs=1, space="PSUM") as ops1,
            tc.tile_pool(name="sps1", bufs=1, space="PSUM") as sps1,
            tc.tile_pool(name="yps", bufs=1, space="PSUM") as yps,
        ):
            ot_ps1 = [
                ops1.tile([HEAD_DIM + 1, 512], F32, tag=f"p{h}", name=f"p{h}")
                for h in range(HPC)
            ]
            for mt in range(MT):
                emit_scores(sps1, mt, 1, 0)
                emit_scores(sps1, mt, 1, 1)
                if mt > 0:
                    emit_pv(ot_ps1, mt - 1, 1)
                if mt == 1:
                    for h in range(HPC):
                        normalize(h, 0, yps)
                elif mt in (3, 5, 7, 9):
                    emit_oproj(yps, (mt - 3) // 2, nc.sync)
            emit_pv(ot_ps1, MT - 1, 1)
            for h in range(HPC):
                nc.vector.tensor_copy(ot_sb[:, h, 512:], ot_ps1[h])
            for h in range(HPC):
                normalize(h, 1, yps)
            for nb in range(N // P // 2, N // P):
                emit_oproj(yps, nb, nc.sync if nb % 2 else nc.scalar)

    nc.compile()
    return nc


_NC_CACHE = []


def _get_nc():
    if not _NC_CACHE:
        _NC_CACHE.append(build_program())
    return _NC_CACHE[0]


def make_in_maps(x, context, mask, Wq, Wkv, b_kv, Wo):
    bf = ml_dtypes.bfloat16
    f8 = ml_dtypes.float8_e4m3
    x = np.asarray(x, dtype=np.float32)
    context = np.asarray(context, dtype=np.float32)
    mask = np.asarray(mask)
    Wq = np.asarray(Wq, dtype=np.float32)
    Wkv = np.asarray(Wkv, dtype=np.float32)
    b_kv = np.asarray(b_kv, dtype=np.float32)
    Wo = np.asarray(Wo, dtype=np.float32)
    eye = np.eye(HEAD_DIM, dtype=bf)

    in_maps = []
    for b in range(B):
        xtb = np.ascontiguousarray(x[b].T).astype(bf)
        ctb = np.ascontiguousarray(context[b].T).astype(bf)
        mbb = np.where(mask[b].T, 0.0, MBIAS).astype(f8)
        for g in range(NUM_HEADS // HPC):
            sl = slice(E * g, E * (g + 1))
            in_maps.append(
                {
                    "xT": xtb,
                    "ctxT": ctb,
                    "mb": mbb,
                    "wqT": np.ascontiguousarray(Wq[sl].T).astype(bf),
                    "wkT": np.ascontiguousarray(Wkv[sl].T).astype(bf),
                    "wvT": np.ascontiguousarray(
                        Wkv[DIM + E * g : DIM + E * (g + 1)].T
                    ).astype(bf),
                    "woT": np.ascontiguousarray(Wo[:, sl].T).astype(bf),
                    "bk": np.ascontiguousarray(b_kv[sl]),
                    "eye64": eye,
                }
            )
    return in_maps


def combine_outputs(ys, b_kv, Wo):
    """ys: list of 8 per-core partial outputs [N, DIM], core order (b, g)."""
    b_v = np.asarray(b_kv, dtype=np.float32)[DIM:]
    ybias = np.asarray(Wo, dtype=np.float32) @ b_v  # [DIM]
    out = np.empty((B, N, DIM), dtype=np.float32)
    G = NUM_HEADS // HPC
    for b in range(B):
        acc = np.asarray(ys[G * b], dtype=np.float32)
        for g in range(1, G):
            acc = acc + np.asarray(ys[G * b + g], dtype=np.float32)
        out[b] = acc + ybias[None, :]
    return out


def kernel(x, context, mask, Wq, Wkv, b_kv, Wo):
    nc = _get_nc()
    in_maps = make_in_maps(x, context, mask, Wq, Wkv, b_kv, Wo)
    res = run_bass_kernel_spmd(nc, in_maps, core_ids=list(range(8)))
    ys = [m["y"] for m in res.results]
    return combine_outputs(ys, b_kv, Wo)


# revision 8
# speedup vs baseline: 1.2051x; 1.2051x over previous
"""CrossAttention Trainium2 kernel (8 NeuronCores, SPMD), bf16 compute.

Sharding: data-parallel over batch B=2, tensor-parallel over the 16 heads in
4 groups of 4 heads -> 8 cores, one (batch, head-group) pair each. Each core
computes its 4 heads' Q/K/V projections, masked softmax cross-attention, and
its partial output projection; the host sums the 4 partial outputs per batch
(the Wo row-split all-reduce, done at unshard time) and adds the constant
Wo @ b_v term (softmax rows sum to 1 so it factors out of the attention).

Numerics: bf16 matmuls with fp32 PSUM accumulation; softmax statistics stay
fp32. exp() is unnormalized (no max subtraction; |scores*scale| < ~2.5) and
runs on ACT straight out of PSUM with the softmax scale fused; the 0/1 mask
is applied multiplicatively afterwards on DVE (all-bf16 SBUF operands, 2x
mode). y is written bf16; the host accumulates the partials in fp32.

Layout: activations and weights arrive contraction-major (host
pre-transposed) so every DMA is a contiguous row load; no device transposes.
Attention is scores-transposed: ST[m, n] per head, so PV contracts over m
directly. The softmax denominator comes free from an appended ones-column on
the v stationary operand. Masked exps live in rotating buffers: PV consumes
them one m-tile behind the exp, so nothing is parked in SBUF.

Schedule: two n-half passes over the 16 m-tiles. Per-head score tiles
single-bank in PSUM with a 3-deep rotation keep the PE continuously fed
(never gated on the ACT/DVE chain); the V projection and the last 3/4 of
the K projection ride inside pass 0 (K shares the score-tile rotation), so
only Q and the first K chunk gate the pipeline start. ctx DMAs are issued
quarter-major (all contraction chunks of each m-quarter first) so K chunk 0
lands early. Pass 0's normalize + output projection + y writeback overlap
pass 1; odd-head normalized outputs reach partitions 64:128 via an
identity-stationary PE matmul at column offset 64.
"""

import numpy as np
import ml_dtypes

import concourse.bass as bass
import concourse.bacc as bacc
import concourse.mybir as mybir
import concourse.tile as tile
from concourse.bass_utils import run_bass_kernel_spmd

DIM = 1024
HEAD_DIM = 64
NUM_HEADS = 16
SCALE = HEAD_DIM**-0.5
B, N, M = 2, 1024, 2048
HPC = 4  # heads per core
E = HPC * HEAD_DIM  # 256: per-core projection width
P = 128
F32 = mybir.dt.float32
BF16 = mybir.dt.bfloat16
CT = DIM // P  # 8 contraction tiles
MT = M // P  # 16 m tiles


def _bc_heads(ap, n):
    """Broadcast a [P, F] AP to [P, n, F] with a zero-stride middle dim."""
    return bass.AP(ap.tensor, ap.offset, [ap.ap[0], [0, n], ap.ap[1]])


def _group_heads(ap, hpc, hd):
    """View a [P, hpc*hd] AP as [P, hpc, hd]."""
    assert ap.ap[-1][0] == 1 and ap.ap[-1][1] == hpc * hd
    return bass.AP(ap.tensor, ap.offset, [ap.ap[0], [hd, hpc], [1, hd]])


def build_program():
    nc = bacc.Bacc("TRN2", target_bir_lowering=False, debug=False, num_devices=8)

    # contraction-major inputs (host pre-transposed)
    xT_d = nc.dram_tensor("xT", [DIM, N], BF16, kind="ExternalInput").ap()
    ctxT_d = nc.dram_tensor("ctxT", [DIM, M], BF16, kind="ExternalInput").ap()
    mk_d = nc.dram_tensor("mk", [M, N], BF16, kind="ExternalInput").ap()
    wqT_d = nc.dram_tensor("wqT", [DIM, E], BF16, kind="ExternalInput").ap()
    wkT_d = nc.dram_tensor("wkT", [DIM, E], BF16, kind="ExternalInput").ap()
    wvT_d = nc.dram_tensor("wvT", [DIM, E], BF16, kind="ExternalInput").ap()
    woT_d = nc.dram_tensor("woT", [E, DIM], BF16, kind="ExternalInput").ap()
    bk_d = nc.dram_tensor("bk", [E], F32, kind="ExternalInput").ap()
    eye_d = nc.dram_tensor(
        "eye64", [HEAD_DIM, HEAD_DIM], BF16, kind="ExternalInput"
    ).ap()
    y_d = nc.dram_tensor("y", [N, DIM], BF16, kind="ExternalOutput").ap()

    Exp = mybir.ActivationFunctionType.Exp

    from contextlib import ExitStack

    with tile.TileContext(nc) as tc, ExitStack() as ctx:
        const = ctx.enter_context(tc.tile_pool(name="const", bufs=1))
        bk_sb = const.tile([P, E // P], F32)
        eye64 = const.tile([HEAD_DIM, HEAD_DIM], BF16)
        nc.gpsimd.dma_start(out=bk_sb, in_=bk_d.rearrange("(t p) -> p t", p=P))
        nc.gpsimd.dma_start(out=eye64, in_=eye_d)

        persist = ctx.enter_context(tc.tile_pool(name="persist", bufs=1))
        qT = persist.tile([P, E // P, N], BF16)
        kT = persist.tile([P, E // P, M], BF16)
        vaug = persist.tile([P, MT, HPC, HEAD_DIM + 1], BF16)
        woT = persist.tile([P, E // P, DIM], BF16)
        mask = persist.tile([P, MT, N], BF16)
        ot_sb = persist.tile([HEAD_DIM + 1, HPC, N], F32)  # PV accumulator park
        otn2 = persist.tile([P, E // P, N], BF16)  # normalized attn out

        # ones column for the softmax denominator; v evictions fill 0:64
        nc.vector.memset(vaug, 1.0)

        expl = ctx.enter_context(tc.tile_pool(name="expl", bufs=3))
        exml = ctx.enter_context(tc.tile_pool(name="exml", bufs=3))
        dnp = ctx.enter_context(tc.tile_pool(name="dnp", bufs=2))
        rbp = ctx.enter_context(tc.tile_pool(name="rbp", bufs=3))
        ypool = ctx.enter_context(tc.tile_pool(name="ypool", bufs=3))

        ex_tiles = {}

        def emit_scores(spool, mt, chn, hp):
            """Per-head bf16 scores -> ACT exp (PSUM read, scale fused)."""
            for hl in range(2):
                h = hp * 2 + hl
                st = spool.tile([P, 512], F32, tag="st", name="st", bufs=3)
                dr = slice(hl * HEAD_DIM, (hl + 1) * HEAD_DIM)
                nc.tensor.matmul(
                    st,
                    lhsT=kT[dr, hp, mt * P : (mt + 1) * P],
                    rhs=qT[dr, hp, chn * 512 : (chn + 1) * 512],
                    start=True,
                    stop=True,
                )
                ex = ex_tiles[(mt, chn)]
                nc.scalar.activation(ex[:, h, :], st, Exp, scale=float(SCALE))

        def emit_mask_mul(mt, chn):
            """One DVE 2x multiply masks all 4 heads of (mt, chn)."""
            ex = ex_tiles[(mt, chn)]
            exm = exml.tile([P, HPC, 512], BF16, tag="exm", name="exm")
            mks = _bc_heads(mask[:, mt, chn * 512 : (chn + 1) * 512], HPC)
            nc.vector.tensor_mul(exm, ex, mks)
            ex_tiles[(mt, chn)] = exm  # PV reads the masked version

        def emit_pv(ot_ps, mt, chn):
            exm = ex_tiles.pop((mt, chn))
            for h in range(HPC):
                nc.tensor.matmul(
                    ot_ps[h],
                    lhsT=vaug[:, mt, h, :],
                    rhs=exm[:, h, :],
                    start=(mt == 0),
                    stop=(mt == MT - 1),
                )

        def emit_step(spool, ot_ps, mt, chn):
            """scores+exp for all heads of (mt,chn), mask-mul, PV(mt-1)."""
            ex_tiles[(mt, chn)] = expl.tile([P, HPC, 512], BF16, tag="ex", name="ex")
            emit_scores(spool, mt, chn, 0)
            yield  # pass-specific PE filler slot (V proj / K proj / O proj)
            emit_scores(spool, mt, chn, 1)
            emit_mask_mul(mt, chn)
            if mt > 0:
                emit_pv(ot_ps, mt - 1, chn)
            yield

        # ---------------- input DMAs ---------------------------------------
        with tc.tile_pool(name="wx", bufs=1) as wx_pool:
            wqT = wx_pool.tile([P, CT, E], BF16)
            xT = wx_pool.tile([P, CT, N], BF16)
            wkT = wx_pool.tile([P, CT, E], BF16)
            wvT = wx_pool.tile([P, CT, E], BF16)
            ctxT = wx_pool.tile([P, CT, M], BF16)

            # sync ring: wq, x (Q path), mask first half
            nc.sync.dma_start(out=wqT, in_=wqT_d.rearrange("(c p) e -> p c e", p=P))
            for j in range(CT):
                nc.sync.dma_start(out=xT[:, j, :], in_=xT_d[j * P : (j + 1) * P, :])
            for mt in range(8):
                nc.sync.dma_start(
                    out=mask[:, mt, :], in_=mk_d[mt * P : (mt + 1) * P, :]
                )
            # scalar ring: wk, ctx m-quarters 0-2 (all j per quarter)
            nc.scalar.dma_start(out=wkT, in_=wkT_d.rearrange("(c p) e -> p c e", p=P))
            for q in range(3):
                for j in range(CT):
                    nc.scalar.dma_start(
                        out=ctxT[:, j, q * 512 : (q + 1) * 512],
                        in_=ctxT_d[j * P : (j + 1) * P, q * 512 : (q + 1) * 512],
                    )
            # gpsimd ring: wv, ctx last m-quarter, mask second half, wo
            nc.gpsimd.dma_start(out=wvT, in_=wvT_d.rearrange("(c p) e -> p c e", p=P))
            for j in range(CT):
                nc.gpsimd.dma_start(
                    out=ctxT[:, j, 1536:], in_=ctxT_d[j * P : (j + 1) * P, 1536:]
                )
            for mt in range(8, MT):
                nc.gpsimd.dma_start(
                    out=mask[:, mt, :], in_=mk_d[mt * P : (mt + 1) * P, :]
                )
            nc.gpsimd.dma_start(out=woT, in_=woT_d.rearrange("(c p) e -> p c e", p=P))

            # ---------------- Q projection (x-gated) -----------------------
            with tc.tile_pool(name="qps", bufs=3, space="PSUM") as qps:
                for et in range(E // P):
                    for chn in range(N // 512):
                        pq = qps.tile([P, 512], F32, tag="pq")
                        for j in range(CT):
                            nc.tensor.matmul(
                                pq,
                                lhsT=wqT[:, j, et * P : (et + 1) * P],
                                rhs=xT[:, j, chn * 512 : (chn + 1) * 512],
                                start=(j == 0),
                                stop=(j == CT - 1),
                            )
                        nc.vector.tensor_copy(
                            qT[:, et, chn * 512 : (chn + 1) * 512], pq
                        )

            # ---------------- pass 0 with V/K projections inline -----------
            with (
                tc.tile_pool(name="sps0", bufs=1, space="PSUM") as sps0,
                tc.tile_pool(name="vps", bufs=1, space="PSUM") as vps,
            ):

                def emit_kproj(et, chm):
                    # shares the score-tile single-bank rotation (tag "st")
                    pk = sps0.tile([P, 512], F32, tag="st", name="pk", bufs=3)
                    for j in range(CT):
                        nc.tensor.matmul(
                            pk,
                            lhsT=wkT[:, j, et * P : (et + 1) * P],
                            rhs=ctxT[:, j, chm * 512 : (chm + 1) * 512],
                            start=(j == 0),
                            stop=(j == CT - 1),
                        )
                    nc.vector.tensor_scalar_add(
                        kT[:, et, chm * 512 : (chm + 1) * 512],
                        pk,
                        bk_sb[:, et : et + 1],
                    )

                def emit_vproj(mt):
                    pv = vps.tile([P, E], F32, tag="pv", name="pv")
                    for j in range(CT):
                        nc.tensor.matmul(
                            pv,
                            lhsT=ctxT[:, j, mt * P : (mt + 1) * P],
                            rhs=wvT[:, j, :],
                            start=(j == 0),
                            stop=(j == CT - 1),
                        )
                    nc.vector.tensor_copy(
                        vaug[:, mt, :, :HEAD_DIM],
                        _group_heads(pv[:, :], HPC, HEAD_DIM),
                    )

                emit_kproj(0, 0)
                emit_kproj(1, 0)
                emit_vproj(0)

                kfill = {0: (0, 1), 1: (1, 1), 4: (0, 2), 5: (1, 2),
                         8: (0, 3), 9: (1, 3)}
                with tc.tile_pool(name="ops0", bufs=1, space="PSUM") as ops0:
                    ot_ps0 = [
                        ops0.tile([HEAD_DIM + 1, 512], F32, tag=f"o{h}", name=f"o{h}")
                        for h in range(HPC)
                    ]
                    for mt in range(MT):
                        step = emit_step(sps0, ot_ps0, mt, 0)
                        next(step)
                        if mt < MT - 1:
                            emit_vproj(mt + 1)
                        next(step, None)
                        if mt in kfill:
                            emit_kproj(*kfill[mt])
                    emit_pv(ot_ps0, MT - 1, 0)
                    for h in range(HPC):
                        nc.vector.tensor_copy(ot_sb[:, h, :512], ot_ps0[h])

        def normalize_front(h, chn):
            """softmax-normalize head h's n-half chn from the ot_sb park.
            Even heads land in otn2 directly; odd heads return a bf16 tmp
            that normalize_shift moves to partitions 64:128 later (keeps the
            PE-queue identity matmul decoupled from this DVE/gpsimd chain)."""
            cs = slice(chn * 512, (chn + 1) * 512)
            hp, hl = divmod(h, 2)
            dn0 = dnp.tile([1, 512], F32, tag="dn", name="dn")
            # row 64 (denominator) -> partition 0 via SBUF-SBUF DMA
            nc.sync.dma_start(out=dn0, in_=ot_sb[HEAD_DIM : HEAD_DIM + 1, h, cs])
            rc = rbp.tile([1, 512], F32, tag="rc", name="rc")
            nc.vector.reciprocal_approx_fast(out=rc, in_=dn0)
            rb = rbp.tile([HEAD_DIM, 512], F32, tag="rb", name="rb")
            nc.gpsimd.partition_broadcast(rb, rc)
            if hl == 0:
                nc.vector.tensor_mul(
                    otn2[:HEAD_DIM, hp, cs], ot_sb[:HEAD_DIM, h, cs], rb
                )
                return None
            tmp = rbp.tile([HEAD_DIM, 512], BF16, tag="tmp", name="tmp")
            nc.vector.tensor_mul(tmp, ot_sb[:HEAD_DIM, h, cs], rb)
            return tmp

        def normalize_shift(h, chn, tmp, yps):
            """odd-head normalized out -> partitions 64:128 via identity mm."""
            cs = slice(chn * 512, (chn + 1) * 512)
            hp = h // 2
            sh = yps.tile([P, 512], F32, tag="yp", name="sh")
            nc.tensor.matmul(
                sh[HEAD_DIM:P, :], lhsT=eye64, rhs=tmp, start=True, stop=True
            )
            nc.vector.tensor_copy(otn2[HEAD_DIM:P, hp, cs], sh[HEAD_DIM:P, :])

        def emit_oproj(yps, nb, ring):
            for oc in range(DIM // 512):
                yp = yps.tile([P, 512], F32, tag="yp", name="yp")
                for hp in range(E // P):
                    nc.tensor.matmul(
                        yp,
                        lhsT=otn2[:, hp, nb * P : (nb + 1) * P],
                        rhs=woT[:, hp, oc * 512 : (oc + 1) * 512],
                        start=(hp == 0),
                        stop=(hp == E // P - 1),
                    )
                ys = ypool.tile([P, 512], BF16, tag="ys", name="ys")
                nc.vector.tensor_copy(ys, yp)
                ring.dma_start(
                    out=y_d[nb * P : (nb + 1) * P, oc * 512 : (oc + 1) * 512], in_=ys
                )

        # ---------------- pass 1: n-cols 512:1024 + pass-0 tail work -------
        with (
            tc.tile_pool(name="sps1", bufs=1, space="PSUM") as sps1,
            tc.tile_pool(name="ops1", bufs=1, space="PSUM") as ops1,
            tc.tile_pool(name="yps", bufs=1, space="PSUM") as yps,
        ):
            ot_ps1 = [
                ops1.tile([HEAD_DIM + 1, 512], F32, tag=f"p{h}", name=f"p{h}")
                for h in range(HPC)
            ]
            tmps = {}
            for mt in range(MT):
                step = emit_step(sps1, ot_ps1, mt, 1)
                next(step)
                if mt in (4, 6, 8, 10):
                    emit_oproj(yps, (mt - 4) // 2, nc.sync)
                next(step, None)
                if mt == 0:
                    for h in range(HPC):
                        tmps[h] = normalize_front(h, 0)
                elif mt == 2:
                    for h in (1, 3):
                        normalize_shift(h, 0, tmps[h], yps)
            emit_pv(ot_ps1, MT - 1, 1)
            for h in range(HPC):
                nc.vector.tensor_copy(ot_sb[:, h, 512:], ot_ps1[h])
            for h in range(HPC):
                tmps[h] = normalize_front(h, 1)
            for h in (1, 3):
                normalize_shift(h, 1, tmps[h], yps)
            for nb in range(N // P // 2, N // P):
                emit_oproj(yps, nb, nc.sync if nb % 2 else nc.scalar)

    nc.compile()
    return nc


_NC_CACHE = []


def _get_nc():
    if not _NC_CACHE:
        _NC_CACHE.append(build_program())
    return _NC_CACHE[0]


def make_in_maps(x, context, mask, Wq, Wkv, b_kv, Wo):
    bf = ml_dtypes.bfloat16
    x = np.asarray(x, dtype=np.float32)
    context = np.asarray(context, dtype=np.float32)
    mask = np.asarray(mask)
    Wq = np.asarray(Wq, dtype=np.float32)
    Wkv = np.asarray(Wkv, dtype=np.float32)
    b_kv = np.asarray(b_kv, dtype=np.float32)
    Wo = np.asarray(Wo, dtype=np.float32)
    eye = np.eye(HEAD_DIM, dtype=bf)

    in_maps = []
    for b in range(B):
        xtb = np.ascontiguousarray(x[b].T).astype(bf)
        ctb = np.ascontiguousarray(context[b].T).astype(bf)
        mkb = np.ascontiguousarray(mask[b].T).astype(bf)
        for g in range(NUM_HEADS // HPC):
            sl = slice(E * g, E * (g + 1))
            in_maps.append(
                {
                    "xT": xtb,
                    "ctxT": ctb,
                    "mk": mkb,
                    "wqT": np.ascontiguousarray(Wq[sl].T).astype(bf),
                    "wkT": np.ascontiguousarray(Wkv[sl].T).astype(bf),
                    "wvT": np.ascontiguousarray(
                        Wkv[DIM + E * g : DIM + E * (g + 1)].T
                    ).astype(bf),
                    "woT": np.ascontiguousarray(Wo[:, sl].T).astype(bf),
                    "bk": np.ascontiguousarray(b_kv[sl]),
                    "eye64": eye,
                }
            )
    return in_maps


def combine_outputs(ys, b_kv, Wo):
    """ys: list of 8 per-core partial outputs [N, DIM], core order (b, g)."""
    b_v = np.asarray(b_kv, dtype=np.float32)[DIM:]
    ybias = np.asarray(Wo, dtype=np.float32) @ b_v  # [DIM]
    out = np.empty((B, N, DIM), dtype=np.float32)
    G = NUM_HEADS // HPC
    for b in range(B):
        acc = np.asarray(ys[G * b], dtype=np.float32)
        for g in range(1, G):
            acc = acc + np.asarray(ys[G * b + g], dtype=np.float32)
        out[b] = acc + ybias[None, :]
    return out


def kernel(x, context, mask, Wq, Wkv, b_kv, Wo):
    nc = _get_nc()
    in_maps = make_in_maps(x, context, mask, Wq, Wkv, b_kv, Wo)
    res = run_bass_kernel_spmd(nc, in_maps, core_ids=list(range(8)))
    ys = [m["y"] for m in res.results]
    return combine_outputs(ys, b_kv, Wo)
